# revision 4
# baseline (speedup 1.0000x reference)
"""BERT-embedding kernel for Trainium2 (8 NeuronCores, data-parallel).

Computes, for input_sequence [256,512,10], doy_sequence [256,512] (int32),
W [256,10], b [256]:

    obs = input_sequence @ W.T + b          # [256,512,256]
    pos = PE_TABLE[doy_sequence]            # [256,512,256]
    out = concat([obs, pos], axis=-1)       # [256,512,512] fp32

Strategy: shard the batch dim 8 ways (32 batches / 16384 tokens per core),
replicate W/b and the 367x256 sinusoidal PE table. The kernel is HBM-write
bound (33.5 MB of output per core), so the design spends no extra HBM
traffic beyond ~0.9 MB of inputs:

  - The PE table lives in SBUF as fp16 [128, 3*256] (row r = 128*k + p).
  - pos rows are produced by a one-hot matmul instead of a DMA gather:
    gpsimd partition_broadcast replicates doy (fp16) across all 128
    partitions, the vector engine compares it against per-partition row
    ids (is_equal, fp16 in/out -> 2x DVE rate) to build onehot^T
    [row, token] fp16, and three accumulated fp16 matmuls per 128-token
    group compute onehot^T.T @ pe. Chunk 0's broadcast ships from the
    host so nothing waits on the gpsimd library IRAM load.
  - obs is one K=11 fp16 matmul ([x^T; ones] . [W^T; b]) into the same
    PSUM pair-tile (plain fp16 is ~1e-3 absolute error, far inside the
    2e-2 gate; fp16 halves the x upload vs fp32).
  - PSUM is used as 4 pair-tiles [128, 2*512] (2 banks each): one
    PSUM->SBUF copy per 256 tokens (vector/scalar engines alternating),
    one token-major 2 MB HWDGE DMA per 1024-token chunk writing rows as
    contiguous 2 KB descriptors at full DMA-bus efficiency.

vs. the SWDGE-gather version this removes 16.8 MB of random 1KB HBM
reads and all Q7 descriptor generation; per-core DMA is ~34 MB ~= the
output-write roofline. vs. the first one-hot version it keeps the PE
dependency-free within a chunk (p-state ramp) and halves DVE work.
"""

import math

import numpy as np

import concourse.bacc as bacc
import concourse.mybir as mybir
import concourse.tile as tile
from concourse.bass_utils import run_bass_kernel_spmd
from concourse.library_config import mlp

F32 = mybir.dt.float32
F16 = mybir.dt.float16

# Problem shapes (hardcoded per the harness contract).
B, S, NF = 256, 512, 10
E = 256
MAX_LEN = 366
N_CORES = 8
TOK = (B // N_CORES) * S          # tokens per core = 16384
CH = 1024                          # tokens per chunk
GRP = CH // 128                    # 128-token groups per chunk = 8
NPAIR = GRP // 2                   # PSUM pair-tiles per chunk = 4
NCH = TOK // CH                    # 16
KF = NF + 1                        # obs contraction dim (features + bias row)
NKC = 3                            # one-hot row chunks (384 padded rows)

_COMPILED_NC = None
_LAST_RESULTS = None               # BassKernelResults of the most recent run


def _make_pe() -> np.ndarray:
    """Sinusoidal table, row 0 zeros (padding), rows 1..366 = positions 0..365."""
    pe = np.zeros((128 * NKC, E), dtype=np.float32)
    position = np.arange(0, MAX_LEN, dtype=np.float32)[:, None]
    div_term = np.exp(
        np.arange(0, E, 2, dtype=np.float32) * -(math.log(10000.0) / E)
    )
    pe[1 : MAX_LEN + 1, 0::2] = np.sin(position * div_term)
    pe[1 : MAX_LEN + 1, 1::2] = np.cos(position * div_term)
    return pe


def _build():
    nc = bacc.Bacc("TRN2", target_bir_lowering=False, debug=False)
    xT = nc.dram_tensor("xT", [KF, TOK], F16, kind="ExternalInput")
    wT = nc.dram_tensor("wT", [KF, E], F16, kind="ExternalInput")
    pe3 = nc.dram_tensor("pe3", [128, NKC * E], F16, kind="ExternalInput")
    doy = nc.dram_tensor("doy", [1, TOK], F16, kind="ExternalInput")
    doyb0 = nc.dram_tensor("doyb0", [128, CH], F16, kind="ExternalInput")
    cmp = nc.dram_tensor("cmp", [128, NKC], F32, kind="ExternalInput")
    out = nc.dram_tensor("out", [TOK, 2 * E], F32, kind="ExternalOutput")

    # out viewed as [chunk, partition, group, 512]: token (c*GRP+j)*128+p
    out4 = out.ap().rearrange("(c j p) e -> c p j e", p=128, j=GRP)

    with tile.TileContext(nc) as tc:
        with (
            tc.tile_pool(name="const", bufs=1) as const_pool,
            tc.tile_pool(name="doyb", bufs=3) as doyb_pool,
            tc.tile_pool(name="oh", bufs=3) as oh_pool,
            tc.tile_pool(name="ot", bufs=3) as ot_pool,
            tc.tile_pool(name="ps", bufs=4, space="PSUM") as ps_pool,
        ):
            # Q7 ucode for partition_broadcast; IRAM DMA overlaps the loads.
            nc.gpsimd.load_library(mlp)

            wT_sb = const_pool.tile([KF, E], F16, tag="wT_sb")
            nc.scalar.dma_start(out=wT_sb[:], in_=wT[:, :])
            pe_sb = const_pool.tile([128, NKC * E], F16, tag="pe_sb")
            nc.scalar.dma_start(out=pe_sb[:], in_=pe3[:, :])
            cmp_sb = const_pool.tile([128, NKC], F32, tag="cmp_sb")
            nc.scalar.dma_start(out=cmp_sb[:], in_=cmp[:, :])
            doyb0_sb = const_pool.tile([128, CH], F16, tag="doyb0_sb")
            nc.scalar.dma_start(out=doyb0_sb[:], in_=doyb0[:, :])
            doy_sb = const_pool.tile([1, TOK], F16, tag="doy_sb")
            nc.sync.dma_start(out=doy_sb[:], in_=doy[:, :])
            xT_sb = const_pool.tile([KF, TOK], F16, tag="xT_sb")
            # Chunked loads so early matmuls start before the full load lands.
            for q in range(4):
                nc.sync.dma_start(
                    out=xT_sb[:, q * (TOK // 4) : (q + 1) * (TOK // 4)],
                    in_=xT[:, q * (TOK // 4) : (q + 1) * (TOK // 4)],
                )

            for c in range(NCH):
                # doyb[p, t] = doy[t] for the chunk's tokens, fp16.
                if c == 0:
                    doyb = doyb0_sb
                else:
                    doyb = doyb_pool.tile([128, CH], F16, tag="doyb")
                    nc.gpsimd.partition_broadcast(
                        doyb[:], doy_sb[0:1, c * CH : (c + 1) * CH]
                    )
                # onehot^T[p, k, t] = (doy[t] == 128*k + p), fp16 at 2x rate.
                oh = oh_pool.tile([128, NKC, CH], F16, tag="oh")
                for k in range(NKC):
                    nc.vector.tensor_scalar(
                        out=oh[:, k, :],
                        in0=doyb[:],
                        scalar1=cmp_sb[:, k : k + 1],
                        scalar2=None,
                        op0=mybir.AluOpType.is_equal,
                    )

                ot = ot_pool.tile([128, GRP, 2 * E], F32, tag="ot")
                for p in range(NPAIR):
                    ps = ps_pool.tile([128, 2, 2 * E], F32, tag="ps")
                    for g in range(2):
                        j = p * 2 + g
                        t0 = c * CH + j * 128
                        nc.tensor.matmul(
                            out=ps[:, g, 0:E],
                            lhsT=xT_sb[:, t0 : t0 + 128],
                            rhs=wT_sb[:],
                            start=True,
                            stop=True,
                        )
                        for k in range(NKC):
                            nc.tensor.matmul(
                                out=ps[:, g, E : 2 * E],
                                lhsT=oh[:, k, j * 128 : (j + 1) * 128],
                                rhs=pe_sb[:, k * E : (k + 1) * E],
                                start=(k == 0),
                                stop=(k == NKC - 1),
                            )
                    # One PSUM->SBUF copy per pair; alternate engines.
                    if p % 2 == 0:
                        nc.vector.tensor_copy(
                            out=ot[:, p * 2 : p * 2 + 2, :], in_=ps[:]
                        )
                    else:
                        nc.scalar.activation(
                            out=ot[:, p * 2 : p * 2 + 2, :],
                            in_=ps[:],
                            func=mybir.ActivationFunctionType.Copy,
                        )
                eng = nc.sync if c % 2 == 0 else nc.scalar
                eng.dma_start(out=out4[c], in_=ot[:])
    nc.compile()
    return nc


def kernel(input_sequence, doy_sequence, W, b) -> np.ndarray:
    global _COMPILED_NC, _LAST_RESULTS

    x = np.asarray(input_sequence, dtype=np.float32)
    doy = np.asarray(doy_sequence, dtype=np.int32)
    W = np.asarray(W, dtype=np.float32)
    bias = np.asarray(b, dtype=np.float32)

    if _COMPILED_NC is None:
        _COMPILED_NC = _build()
    nc = _COMPILED_NC

    # Augmented weights [11, E]: rows 0..9 = W.T, row 10 = bias (ones-row).
    wTf = np.concatenate([W.T, bias[None, :]], axis=0)
    wT = np.ascontiguousarray(wTf.astype(np.float16))

    # PE table fp16, packed [128, 3*256]: row r=128k+p at pe3[p, k*256:...].
    petab = _make_pe().astype(np.float16)
    pe3 = np.ascontiguousarray(
        petab.reshape(NKC, 128, E).transpose(1, 0, 2).reshape(128, NKC * E)
    )

    # Per-partition compare constants: cmp[p, k] = p + 128k.
    cmpc = np.ascontiguousarray(
        np.arange(128, dtype=np.float32)[:, None]
        + 128.0 * np.arange(NKC, dtype=np.float32)[None, :]
    )

    bpc = B // N_CORES
    in_maps = []
    for c in range(N_CORES):
        xc = x[c * bpc : (c + 1) * bpc].reshape(TOK, NF)
        xTf = np.empty((KF, TOK), dtype=np.float16)
        xTf[:NF] = xc.T.astype(np.float16)
        xTf[NF] = 1.0
        doy_c = doy[c * bpc : (c + 1) * bpc].reshape(1, TOK).astype(np.float16)
        doyb0_c = np.ascontiguousarray(np.broadcast_to(doy_c[:, :CH], (128, CH)))
        in_maps.append(
            {
                "xT": np.ascontiguousarray(xTf),
                "wT": wT,
                "pe3": pe3,
                "doy": np.ascontiguousarray(doy_c),
                "doyb0": doyb0_c,
                "cmp": cmpc,
            }
        )

    _LAST_RESULTS = run_bass_kernel_spmd(nc, in_maps, core_ids=list(range(N_CORES)))

    out = np.empty((B, S, 2 * E), dtype=np.float32)
    for c in range(N_CORES):
        out[c * bpc : (c + 1) * bpc] = _LAST_RESULTS.results[c]["out"].reshape(
            bpc, S, 2 * E
        )
    return out
